# revision 13
# baseline (speedup 1.0000x reference)
"""Trainium2 Bass kernel for a 1-layer transformer encoder (v2).

Reference model (B=32, S=512, D=768, H=12, hd=64, hidden=3072):
    q,k,v = x@Wq, x@Wk, x@Wv         (per head)
    attn  = softmax(q k^T / 8) v
    mha   = concat_heads @ Wo
    out1  = x + LN(mha)
    ffn   = gelu(out1@W1 + b1) @ W2 + b2
    out   = out1 + LN(ffn)

Sharding: data-parallel over batch -- each of the 8 cores gets 4 full
sequences (2048 tokens) and all weights; no collectives.

v2 changes vs the 630us baseline:
  * x arrives BOTH as [T,D] f32 (residual) and pre-transposed bf16
    [D,T] from the host -- no on-device transposes/casts for x.
  * Attention is interleaved into the q/k projection m-loop so the
    ~100us of ACT exp overlaps projection matmuls instead of
    serializing after them.
  * Score matmuls 2-head row-tiled (K=64 pairs at row groups 0-1/2-3)
    -> ~2x score throughput.
  * LN rstd via DVE bit-trick rsqrt (no ACT Sqrt -> only 2 ACT table
    sets ever loaded: Exp, Gelu).
  * out1 kept in SBUF as bf16 (no DRAM spill round-trip).
  * out1 -> o1T transposes on the DMA XBAR (dma_start_transpose), not
    the PE.
  * Softmax denominator reciprocal broadcast per (head-pair, batch)
    inline via small DRAM bounce.
"""

import os
import sys

import numpy as np

for _p in ("/opt/trn_rl_repo", "/root/.axon_site/_ro/trn_rl_repo"):
    if os.path.isdir(_p) and _p not in sys.path:
        sys.path.insert(0, _p)

os.environ.setdefault("MYCRO_LOCAL_CACHE", "1")

import concourse.bacc as bacc
import concourse.tile as tile
from concourse import mybir
from concourse.bass_utils import run_bass_kernel_spmd

F32 = mybir.dt.float32
BF16 = mybir.dt.bfloat16
I32 = mybir.dt.int32
AF = mybir.ActivationFunctionType
OP = mybir.AluOpType

# model dims
D, NHEAD, HD, FF, SEQ, P = 768, 12, 64, 3072, 512, 128
ND = D // P    # 6 feature chunks
NF = FF // P   # 24 hidden chunks
EPS = 1e-5
N_CORES = 8
B_TOTAL = 32
MAGIC1 = 0x5F3759E0  # 0x5f3759df + 1 (for the ~x + (magic+1) form)


def _rsqrt_dve(nc, s, var_ap, out_ap, n):
    """out[:, :n] = 1/sqrt(var[:, :n] + EPS) via bit-trick + 2 Newton steps.

    `s` is a dict of preallocated [P, >=n] scratch tiles:
    veps(F32), ti(I32), yi(I32), ya(F32), p(F32).
    """
    veps = s["veps"][:, 0:n]
    ti = s["ti"][:, 0:n]
    yi = s["yi"][:, 0:n]
    ya = s["ya"][:, 0:n]
    p = s["p"][:, 0:n]
    nc.vector.tensor_scalar(out=veps, in0=var_ap, scalar1=EPS, scalar2=None,
                            op0=OP.add)
    # ti = ~(bits(veps) >> 1)
    nc.vector.tensor_scalar(out=ti, in0=veps.bitcast(I32), scalar1=1,
                            scalar2=-1, op0=OP.logical_shift_right,
                            op1=OP.bitwise_xor)
    # yi = ti + (magic+1)  == magic - (bits >> 1)
    nc.vector.tensor_scalar(out=yi, in0=ti, scalar1=MAGIC1, scalar2=None,
                            op0=OP.add)
    y = yi.bitcast(F32)
    for it in range(2):
        dst = ya if it == 0 else out_ap
        nc.vector.tensor_tensor(out=p, in0=y, in1=y, op=OP.mult)
        nc.vector.tensor_tensor(out=p, in0=p, in1=veps, op=OP.mult)
        nc.vector.tensor_scalar(out=p, in0=p, scalar1=-0.5, scalar2=1.5,
                                op0=OP.mult, op1=OP.add)
        nc.vector.tensor_tensor(out=dst, in0=y, in1=p, op=OP.mult)
        y = dst


def emit(nc, tc, io, bpc):
    T = bpc * SEQ
    NT = T // P          # 16 token chunks
    NB = bpc             # 4 batches per core
    VW = NHEAD * (HD + 1)

    # ---------------- pools ----------------
    consts = tc.alloc_tile_pool(name="consts", bufs=1)
    psp = tc.alloc_tile_pool(name="psp", bufs=1, space="PSUM")
    dramp = tc.alloc_tile_pool(name="dramp", bufs=1, space="DRAM")

    # right stack (reverse-release order): oTp, qkvp, ascr, xtp, wqkv
    oTp = tc.alloc_tile_pool(name="oTp", bufs=1, side="right")
    qkvp = tc.alloc_tile_pool(name="qkvp", bufs=1, side="right")
    ascr = tc.alloc_tile_pool(name="ascr", bufs=1, side="right")
    xtp = tc.alloc_tile_pool(name="xtp", bufs=1, side="right")
    wqkv = tc.alloc_tile_pool(name="wqkv", bufs=1, side="right")

    # ---------------- constants ----------------
    ones1 = consts.tile([1, P], BF16)
    nc.vector.memset(ones1, 1.0)
    b2r = consts.tile([1, D], BF16)
    nc.sync.dma_start(b2r, io["b2r"])
    b1t = consts.tile([P, NF], F32)
    nc.sync.dma_start(b1t, io["b1t"])
    ln2w = consts.tile([P, D], F32, tag="ln2w", name="ln2w_bc")
    nc.gpsimd.dma_start(ln2w, io["ln2w"].broadcast_to([P, D]))
    ln2b = consts.tile([P, D], F32, tag="ln2b", name="ln2b_bc")
    nc.gpsimd.dma_start(ln2b, io["ln2b"].broadcast_to([P, D]))
    wo_sb = []
    for k in range(ND):
        t = consts.tile([P, D], BF16, tag=f"wo{k}", name=f"wo{k}_sb")
        wo_sb.append(t)

    # ---------------- SBUF tensors ----------------
    oT = [oTp.tile([P, T], BF16, tag=f"oT{j}", name=f"oT{j}") for j in range(ND)]
    qT = [qkvp.tile([P, T], BF16, tag=f"qT{j}", name=f"qT{j}") for j in range(ND)]
    kT = [qkvp.tile([P, T], BF16, tag=f"kT{j}", name=f"kT{j}") for j in range(ND)]
    vN = [qkvp.tile([P, VW], BF16, tag=f"v{i}", name=f"v{i}") for i in range(NT)]
    xt = [xtp.tile([P, T], BF16, tag=f"xt{j}", name=f"xt{j}") for j in range(ND)]

    # weight DMAs (sync ring, consumption order: wv, wq, wk, wo)
    wv_sb, wq_sb, wk_sb = [], [], []
    for lst, name in ((wv_sb, "wv"), (wq_sb, "wq"), (wk_sb, "wk")):
        for k in range(ND):
            t = wqkv.tile([P, D], BF16, tag=f"{name}{k}", name=f"{name}{k}_sb")
            lst.append(t)
    for k in range(ND):
        nc.sync.dma_start(wv_sb[k], io["wv"][k * P:(k + 1) * P, :])
    # xT loads (scalar ring), column-group order so v can start early
    for g in range(NB):
        for k in range(ND):
            nc.scalar.dma_start(xt[k][:, g * SEQ:(g + 1) * SEQ],
                                io["xt"][k * P:(k + 1) * P, g * SEQ:(g + 1) * SEQ])
    for k in range(ND):
        nc.sync.dma_start(wq_sb[k], io["wq"][k * P:(k + 1) * P, :])
    for k in range(ND):
        nc.sync.dma_start(wk_sb[k], io["wk"][k * P:(k + 1) * P, :])
    for k in range(ND):
        nc.sync.dma_start(wo_sb[k], io["wo"][k * P:(k + 1) * P, :])

    # attention scratch tiles are allocated per-(m,b) below (tag rotation)

    # ---------------- phase V: v projection ----------------
    for i in range(NT):
        pa = psp.tile([P, SEQ], F32, tag="sc", bufs=4, name="psv_a")
        pb = psp.tile([P, SEQ], F32, tag="qk", bufs=2, name="psv_b")
        for k in range(ND):
            lhs = xt[k][:, i * P:(i + 1) * P]
            nc.tensor.matmul(pa, lhs, wv_sb[k][:, :SEQ],
                             start=(k == 0), stop=(k == ND - 1))
            nc.tensor.matmul(pb[:, 0:D - SEQ], lhs, wv_sb[k][:, SEQ:D],
                             start=(k == 0), stop=(k == ND - 1))
        nc.vector.memset(vN[i][:, HD::HD + 1], 1.0)
        vv = vN[i].rearrange("p (h w) -> p h w", w=HD + 1)
        nc.any.tensor_copy(
            out=vv[:, 0:8, 0:HD],
            in_=pa.rearrange("p (h w) -> p h w", w=HD))
        nc.any.tensor_copy(
            out=vv[:, 8:12, 0:HD],
            in_=pb[:, 0:D - SEQ].rearrange("p (h w) -> p h w", w=HD))

    lnscr = None
    w1p = None
    w1_sb = []

    def attention(mm):
        """Heads 2mm, 2mm+1 for all batches (scores 2-head row-tiled)."""
        for b in range(NB):
            bs = slice(b * SEQ, (b + 1) * SEQ)
            q0 = qT[mm][0:HD, bs]
            q1 = qT[mm][HD:P, bs]
            pts = [[], []]
            for c in range(4):
                s0 = psp.tile([P, SEQ], F32, tag="sc", bufs=4, name="st0")
                s1 = psp.tile([P, SEQ], F32, tag="sc", bufs=4, name="st1")
                kc = slice(b * SEQ + c * P, b * SEQ + (c + 1) * P)
                # row groups 0-1 and 2-3: issued back-to-back -> concurrent
                nc.tensor.matmul(s0, kT[mm][0:HD, kc], q0, start=True, stop=True)
                nc.tensor.matmul(s1, kT[mm][HD:P, kc], q1, start=True, stop=True)
                pt0 = ascr.tile([P, SEQ], BF16, tag=f"pt0_{c}", bufs=2,
                                name=f"pt0_{c}")
                pt1 = ascr.tile([P, SEQ], BF16, tag=f"pt1_{c}", bufs=2,
                                name=f"pt1_{c}")
                nc.scalar.activation(out=pt0, in_=s0, func=AF.Exp)
                nc.scalar.activation(out=pt1, in_=s1, func=AF.Exp)
                pts[0].append(pt0)
                pts[1].append(pt1)
            rb_d = dramp.tile([2, SEQ], BF16, tag="rb_d", bufs=3, name="rb_d")
            for h2 in range(2):
                h = 2 * mm + h2
                hoff = h2 * HD
                ot = psp.tile([HD + 1, SEQ], F32, tag="oc", bufs=2, name="ot")
                for c in range(4):
                    vblk = vN[b * 4 + c][:, h * (HD + 1):(h + 1) * (HD + 1)]
                    nc.tensor.matmul(ot, vblk, pts[h2][c],
                                     start=(c == 0), stop=(c == 3))
                nc.vector.tensor_copy(out=oT[mm][hoff:hoff + HD, bs],
                                      in_=ot[0:HD, :])
                dn = ascr.tile([1, SEQ], F32, tag=f"dn{h2}", bufs=2,
                               name=f"dn{h2}")
                nc.vector.tensor_copy(out=dn, in_=ot[HD:HD + 1, :])
                nc.vector.reciprocal(out=dn, in_=dn)
                rdb = ascr.tile([1, SEQ], BF16, tag=f"rdb{h2}", bufs=2,
                                name=f"rdb{h2}")
                nc.vector.tensor_copy(out=rdb, in_=dn)
                nc.scalar.dma_start(rb_d[h2:h2 + 1, :], rdb)
            # partition-broadcast of both heads' reciprocals via DRAM bounce
            bcs = ascr.tile([P, SEQ], BF16, tag="bcs", bufs=2, name="bcs")
            nc.scalar.dma_start(bcs[0:HD, :],
                                rb_d[0:1, :].broadcast_to([HD, SEQ]))
            nc.scalar.dma_start(bcs[HD:P, :],
                                rb_d[1:2, :].broadcast_to([HD, SEQ]))
            nc.vector.tensor_mul(out=oT[mm][:, bs], in0=oT[mm][:, bs], in1=bcs)

    # ---------------- merged q/k projections + attention ----------------
    for m in range(ND):
        for w_sb, dstT in ((wq_sb, qT), (wk_sb, kT)):
            for gp in range(2):
                pg = [psp.tile([P, SEQ], F32, tag="qk", bufs=2, name=f"pq{g}")
                      for g in range(2)]
                for k in range(ND):
                    lhs = w_sb[k][:, m * P:(m + 1) * P]
                    for g in range(2):
                        gg = gp * 2 + g
                        nc.tensor.matmul(
                            pg[g], lhs, xt[k][:, gg * SEQ:(gg + 1) * SEQ],
                            start=(k == 0), stop=(k == ND - 1))
                for g in range(2):
                    gg = gp * 2 + g
                    nc.any.tensor_copy(
                        out=dstT[m][:, gg * SEQ:(gg + 1) * SEQ], in_=pg[g])
        if m == ND - 1:
            # q/k weights + xt dead after this m's matmuls: free for w1
            wqkv.release()
            xtp.release()
            w1p = tc.alloc_tile_pool(name="w1p", bufs=1)
            for k in range(ND):
                t = w1p.tile([P, FF], BF16, tag=f"w1_{k}", name=f"w1_{k}")
                w1_sb.append(t)
            for k in range(ND):
                nc.sync.dma_start(w1_sb[k], io["w1"][k * P:(k + 1) * P, :])
        if m > 0:
            attention(m - 1)
    attention(ND - 1)

    ascr.release()
    qkvp.release()

    # LN scratch (shared by LN1 and LN2 -- the phases are sequential)
    lnscr = tc.alloc_tile_pool(name="lnscr", bufs=1)
    ln1w = lnscr.tile([P, D], F32, tag="ln1w", name="ln1w_bc")
    nc.gpsimd.dma_start(ln1w, io["ln1w"].broadcast_to([P, D]))
    ln1b = lnscr.tile([P, D], F32, tag="ln1b", name="ln1b_bc")
    nc.gpsimd.dma_start(ln1b, io["ln1b"].broadcast_to([P, D]))
    rs = {
        "veps": lnscr.tile([P, 4], F32, tag="rs_veps", bufs=2, name="rs_veps"),
        "ti": lnscr.tile([P, 4], I32, tag="rs_ti", bufs=2, name="rs_ti"),
        "yi": lnscr.tile([P, 4], I32, tag="rs_yi", bufs=2, name="rs_yi"),
        "ya": lnscr.tile([P, 4], F32, tag="rs_ya", bufs=2, name="rs_ya"),
        "p": lnscr.tile([P, 4], F32, tag="rs_p", bufs=2, name="rs_p"),
    }

    out1p = tc.alloc_tile_pool(name="out1p", bufs=1)
    o1Tp = tc.alloc_tile_pool(name="o1Tp", bufs=1)
    out1_sb = [out1p.tile([P, D], BF16, tag=f"o1_{i}", name=f"o1_{i}")
               for i in range(NT)]
    o1T = [o1Tp.tile([P, T], BF16, tag=f"o1T{j}", name=f"o1T{j}")
           for j in range(ND)]

    def ln_block(pa, pb, mvb, j):
        """bn stats for one [P, D] psum pair into mvb[:, 2j:2j+2]; STT apply
        is done separately after the batched rsqrt."""
        stats = lnscr.tile([P, 3, 6], F32, tag="stats", bufs=2, name="stats")
        nc.vector.bn_stats(out=stats[:, 0, :], in_=pa[:, 0:256])
        nc.vector.bn_stats(out=stats[:, 1, :], in_=pa[:, 256:512])
        nc.vector.bn_stats(out=stats[:, 2, :], in_=pb[:, 0:256])
        nc.vector.bn_aggr(out=mvb[:, 2 * j:2 * j + 2], in_=stats)

    def ln_apply(pa, pb, w_bc, base, mean_ap, rstd_ap, out_tile):
        u = lnscr.tile([P, D], F32, tag="u", bufs=2, name="u")
        nc.vector.scalar_tensor_tensor(
            out=u[:, 0:SEQ], in0=pa, scalar=mean_ap, in1=w_bc[:, 0:SEQ],
            op0=OP.subtract, op1=OP.mult)
        nc.vector.scalar_tensor_tensor(
            out=u[:, SEQ:D], in0=pb[:, 0:D - SEQ], scalar=mean_ap,
            in1=w_bc[:, SEQ:D], op0=OP.subtract, op1=OP.mult)
        nc.vector.scalar_tensor_tensor(
            out=out_tile, in0=u, scalar=rstd_ap, in1=base,
            op0=OP.mult, op1=OP.add)

    # ---------------- Wo + LN1 (per batch, LN sub-batched by 2) ----------------
    mvb1 = lnscr.tile([P, 8], F32, tag="mvb", bufs=2, name="mvb1")
    rstd1 = lnscr.tile([P, 4], F32, tag="rstd", bufs=2, name="rstd1")
    for b in range(NB):
        holds = []
        for j in range(4):
            i = 4 * b + j
            ma = psp.tile([P, SEQ], F32, tag="sc", bufs=4, name="mh_a")
            mb = psp.tile([P, SEQ], F32, tag="qk", bufs=2, name="mh_b")
            for k in range(ND):
                lhs = oT[k][:, i * P:(i + 1) * P]
                nc.tensor.matmul(ma, lhs, wo_sb[k][:, :SEQ],
                                 start=(k == 0), stop=(k == ND - 1))
                nc.tensor.matmul(mb[:, 0:D - SEQ], lhs, wo_sb[k][:, SEQ:D],
                                 start=(k == 0), stop=(k == ND - 1))
            x_t = lnscr.tile([P, D], F32, tag="xin", bufs=3, name="x_t")
            nc.scalar.dma_start(x_t, io["x"][i * P:(i + 1) * P, :])
            xb = lnscr.tile([P, D], F32, tag="resb", bufs=2, name="xb")
            nc.gpsimd.tensor_add(out=xb, in0=x_t, in1=ln1b)
            ln_block(ma, mb, mvb1, j)
            holds.append((i, j, ma, mb, xb))
            if j % 2 == 1:
                _rsqrt_dve(nc, rs, mvb1[:, 2 * (j - 1) + 1:2 * j + 2:2],
                           rstd1[:, j - 1:j + 1], 2)
                for (ii, jj, pma, pmb, pxb) in holds:
                    ln_apply(pma, pmb, ln1w, pxb, mvb1[:, 2 * jj:2 * jj + 1],
                             rstd1[:, jj:jj + 1], out1_sb[ii])
                    for jf in range(ND):
                        nc.scalar.dma_start_transpose(
                            out=o1T[jf][:, ii * P:(ii + 1) * P],
                            in_=out1_sb[ii][:, jf * P:(jf + 1) * P])
                holds = []

    oTp.release()

    # w2 + hidden buffers
    w2p = tc.alloc_tile_pool(name="w2p", bufs=1, side="right")
    w2_sb = [w2p.tile([P, D], BF16, tag=f"w2_{k}", name=f"w2_{k}")
             for k in range(NF)]
    for k in range(NF):
        nc.sync.dma_start(w2_sb[k], io["w2"][k * P:(k + 1) * P, :])
    hbuf = tc.alloc_tile_pool(name="hbuf", bufs=1, side="right")

    # ---------------- FFN + LN2 (per batch) ----------------
    mvb2 = lnscr.tile([P, 8], F32, tag="mvb", bufs=2, name="mvb2")
    rstd2 = lnscr.tile([P, 4], F32, tag="rstd", bufs=2, name="rstd2")
    for b in range(NB):
        gs = slice(b * SEQ, (b + 1) * SEQ)
        hts = []
        for f in range(NF):
            hp = psp.tile([P, SEQ], F32, tag="sc", bufs=4, name="hp")
            for k in range(ND):
                nc.tensor.matmul(hp, w1_sb[k][:, f * P:(f + 1) * P],
                                 o1T[k][:, gs],
                                 start=(k == 0), stop=(k == ND - 1))
            ht = hbuf.tile([P, SEQ], BF16, tag=f"ht{f}", name=f"ht{f}")
            nc.scalar.activation(out=ht, in_=hp, func=AF.Gelu,
                                 bias=b1t[:, f:f + 1], scale=1.0)
            hts.append(ht)
        holds = []
        for j in range(4):
            i = 4 * b + j
            fa = psp.tile([P, SEQ], F32, tag="sc", bufs=4, name="fp_a")
            fb = psp.tile([P, SEQ], F32, tag="qk", bufs=2, name="fp_b")
            for f in range(NF):
                lhs = hts[f][:, j * P:(j + 1) * P]
                nc.tensor.matmul(fa, lhs, w2_sb[f][:, :SEQ],
                                 start=(f == 0), stop=False)
                nc.tensor.matmul(fb[:, 0:D - SEQ], lhs, w2_sb[f][:, SEQ:D],
                                 start=(f == 0), stop=False)
            nc.tensor.matmul(fa, ones1, b2r[:, :SEQ], start=False, stop=True)
            nc.tensor.matmul(fb[:, 0:D - SEQ], ones1, b2r[:, SEQ:D],
                             start=False, stop=True)
            base = lnscr.tile([P, D], F32, tag="resb", bufs=2, name="base")
            nc.gpsimd.tensor_add(out=base, in0=out1_sb[i], in1=ln2b)
            ln_block(fa, fb, mvb2, j)
            holds.append((i, j, fa, fb, base))
            if j % 2 == 1:
                _rsqrt_dve(nc, rs, mvb2[:, 2 * (j - 1) + 1:2 * j + 2:2],
                           rstd2[:, j - 1:j + 1], 2)
                for (ii, jj, pfa, pfb, pbase) in holds:
                    outt = lnscr.tile([P, D], F32, tag="outt", bufs=2,
                                      name="outt")
                    ln_apply(pfa, pfb, ln2w, pbase,
                             mvb2[:, 2 * jj:2 * jj + 1],
                             rstd2[:, jj:jj + 1], outt)
                    nc.scalar.dma_start(io["out"][ii * P:(ii + 1) * P, :], outt)
                holds = []

    hbuf.release()
    w2p.release()
    o1Tp.release()
    out1p.release()
    lnscr.release()
    w1p.release()
    consts.release()
    psp.release()
    dramp.release()


def build(bpc):
    """Build + compile the per-core program. Returns the Bacc object."""
    T = bpc * SEQ
    nc = bacc.Bacc("TRN2", target_bir_lowering=False, debug=False,
                   num_devices=N_CORES)
    io = {
        "x": nc.dram_tensor("x", [T, D], F32, kind="ExternalInput").ap(),
        "xt": nc.dram_tensor("xt", [D, T], BF16, kind="ExternalInput").ap(),
        "wq": nc.dram_tensor("wq", [D, D], BF16, kind="ExternalInput").ap(),
        "wk": nc.dram_tensor("wk", [D, D], BF16, kind="ExternalInput").ap(),
        "wv": nc.dram_tensor("wv", [D, D], BF16, kind="ExternalInput").ap(),
        "wo": nc.dram_tensor("wo", [D, D], BF16, kind="ExternalInput").ap(),
        "w1": nc.dram_tensor("w1", [D, FF], BF16, kind="ExternalInput").ap(),
        "w2": nc.dram_tensor("w2", [FF, D], BF16, kind="ExternalInput").ap(),
        "b1t": nc.dram_tensor("b1t", [P, NF], F32, kind="ExternalInput").ap(),
        "b2r": nc.dram_tensor("b2r", [1, D], BF16, kind="ExternalInput").ap(),
        "ln1w": nc.dram_tensor("ln1w", [1, D], F32, kind="ExternalInput").ap(),
        "ln1b": nc.dram_tensor("ln1b", [1, D], F32, kind="ExternalInput").ap(),
        "ln2w": nc.dram_tensor("ln2w", [1, D], F32, kind="ExternalInput").ap(),
        "ln2b": nc.dram_tensor("ln2b", [1, D], F32, kind="ExternalInput").ap(),
        "out": nc.dram_tensor("out", [T, D], F32, kind="ExternalOutput").ap(),
    }
    with tile.TileContext(nc) as tc:
        emit(nc, tc, io, bpc)
    nc.compile()
    return nc


def prep_weights(inputs):
    """Host-side weight layout prep (numpy only)."""
    bf = mybir.dt.np(BF16)
    f32 = np.float32
    wq = (np.asarray(inputs["Wq"], f32).transpose(1, 0, 2).reshape(D, D)
          / np.sqrt(HD)).astype(bf)
    wk = np.asarray(inputs["Wk"], f32).transpose(1, 0, 2).reshape(D, D).astype(bf)
    wv = np.asarray(inputs["Wv"], f32).transpose(1, 0, 2).reshape(D, D).astype(bf)
    return {
        "wq": np.ascontiguousarray(wq),
        "wk": np.ascontiguousarray(wk),
        "wv": np.ascontiguousarray(wv),
        "wo": np.asarray(inputs["Wo"], f32).astype(bf),
        "w1": np.asarray(inputs["W1"], f32).astype(bf),
        "w2": np.asarray(inputs["W2"], f32).astype(bf),
        "b1t": np.ascontiguousarray(
            np.asarray(inputs["b1"], f32).reshape(NF, P).T),
        "b2r": np.asarray(inputs["b2"], f32).reshape(1, D).astype(bf),
        "ln1w": np.asarray(inputs["ln1_w"], f32).reshape(1, D),
        "ln1b": np.asarray(inputs["ln1_b"], f32).reshape(1, D),
        "ln2w": np.asarray(inputs["ln2_w"], f32).reshape(1, D),
        "ln2b": np.asarray(inputs["ln2_b"], f32).reshape(1, D),
    }


def make_in_maps(inputs):
    """Per-core input dicts (shards x both ways + shared weights)."""
    bf = mybir.dt.np(BF16)
    bpc = B_TOTAL // N_CORES
    w = prep_weights(inputs)
    x = np.asarray(inputs["x"], np.float32)
    in_maps = []
    for c in range(N_CORES):
        shard = np.ascontiguousarray(
            x[c * bpc:(c + 1) * bpc].reshape(bpc * SEQ, D))
        xtr = np.ascontiguousarray(shard.T.astype(bf))
        in_maps.append({"x": shard, "xt": xtr, **w})
    return in_maps


_cache = {}


def kernel(**inputs) -> np.ndarray:
    bpc = B_TOTAL // N_CORES
    if "nc" not in _cache:
        _cache["nc"] = build(bpc)
    nc = _cache["nc"]
    in_maps = make_in_maps(inputs)
    res = run_bass_kernel_spmd(nc, in_maps, list(range(N_CORES)))
    out = np.concatenate(
        [res.results[c]["out"].reshape(bpc, SEQ, D) for c in range(N_CORES)],
        axis=0)
    return np.ascontiguousarray(out.astype(np.float32))


# revision 22
# speedup vs baseline: 1.2311x; 1.2311x over previous
"""Trainium2 Bass kernel for a 1-layer transformer encoder (v2).

Reference model (B=32, S=512, D=768, H=12, hd=64, hidden=3072):
    q,k,v = x@Wq, x@Wk, x@Wv         (per head)
    attn  = softmax(q k^T / 8) v
    mha   = concat_heads @ Wo
    out1  = x + LN(mha)
    ffn   = gelu(out1@W1 + b1) @ W2 + b2
    out   = out1 + LN(ffn)

Sharding: data-parallel over batch -- each of the 8 cores gets 4 full
sequences (2048 tokens) and all weights; no collectives.

v2 changes vs the 630us baseline:
  * x arrives BOTH as [T,D] f32 (residual) and pre-transposed bf16
    [D,T] from the host -- no on-device transposes/casts for x.
  * Attention is interleaved into the q/k projection m-loop so the
    ~100us of ACT exp overlaps projection matmuls instead of
    serializing after them.
  * Score matmuls 2-head row-tiled (K=64 pairs at row groups 0-1/2-3)
    -> ~2x score throughput.
  * LN rstd via DVE bit-trick rsqrt (no ACT Sqrt -> only 2 ACT table
    sets ever loaded: Exp, Gelu).
  * out1 kept in SBUF as bf16 (no DRAM spill round-trip).
  * out1 -> o1T transposes on the DMA XBAR (dma_start_transpose), not
    the PE.
  * Softmax denominator reciprocal broadcast per (head-pair, batch)
    inline via small DRAM bounce.
"""

import os
import sys

import numpy as np

for _p in ("/opt/trn_rl_repo", "/root/.axon_site/_ro/trn_rl_repo"):
    if os.path.isdir(_p) and _p not in sys.path:
        sys.path.insert(0, _p)

os.environ.setdefault("MYCRO_LOCAL_CACHE", "1")

import concourse.bacc as bacc
import concourse.tile as tile
from concourse import mybir
from concourse.bass_utils import run_bass_kernel_spmd
from concourse.masks import make_identity

F32 = mybir.dt.float32
BF16 = mybir.dt.bfloat16
I32 = mybir.dt.int32
AF = mybir.ActivationFunctionType
OP = mybir.AluOpType

# model dims
D, NHEAD, HD, FF, SEQ, P = 768, 12, 64, 3072, 512, 128
ND = D // P    # 6 feature chunks
NF = FF // P   # 24 hidden chunks
EPS = 1e-5
N_CORES = 8
B_TOTAL = 32
MAGIC1 = 0x5F3759E0  # 0x5f3759df + 1 (for the ~x + (magic+1) form)


def _rsqrt_dve(nc, s, var_ap, out_ap, n):
    """out[:, :n] = 1/sqrt(var[:, :n] + EPS) via bit-trick + 2 Newton steps.

    `s` is a dict of preallocated [P, >=n] scratch tiles:
    veps(F32), ti(I32), yi(I32), ya(F32), p(F32).
    """
    veps = s["veps"][:, 0:n]
    ti = s["ti"][:, 0:n]
    yi = s["yi"][:, 0:n]
    ya = s["ya"][:, 0:n]
    p = s["p"][:, 0:n]
    nc.vector.tensor_scalar(out=veps, in0=var_ap, scalar1=EPS, scalar2=None,
                            op0=OP.add)
    # ti = ~(bits(veps) >> 1)
    nc.vector.tensor_scalar(out=ti, in0=veps.bitcast(I32), scalar1=1,
                            scalar2=-1, op0=OP.logical_shift_right,
                            op1=OP.bitwise_xor)
    # yi = ti + (magic+1)  == magic - (bits >> 1)
    nc.vector.tensor_scalar(out=yi, in0=ti, scalar1=MAGIC1, scalar2=None,
                            op0=OP.add)
    y = yi.bitcast(F32)
    for it in range(2):
        dst = ya if it == 0 else out_ap
        nc.vector.tensor_tensor(out=p, in0=y, in1=y, op=OP.mult)
        nc.vector.tensor_tensor(out=p, in0=p, in1=veps, op=OP.mult)
        nc.vector.tensor_scalar(out=p, in0=p, scalar1=-0.5, scalar2=1.5,
                                op0=OP.mult, op1=OP.add)
        nc.vector.tensor_tensor(out=dst, in0=y, in1=p, op=OP.mult)
        y = dst


def emit(nc, tc, io, bpc):
    T = bpc * SEQ
    NT = T // P          # 16 token chunks
    NB = bpc             # 4 batches per core
    VW = NHEAD * (HD + 1)

    # ---------------- pools ----------------
    consts = tc.alloc_tile_pool(name="consts", bufs=1)
    psp = tc.alloc_tile_pool(name="psp", bufs=1, space="PSUM")
    dramp = tc.alloc_tile_pool(name="dramp", bufs=1, space="DRAM")

    # right stack (reverse-release order): oTp, qkvp, ascr, xtp, wqkv
    oTp = tc.alloc_tile_pool(name="oTp", bufs=1, side="right")
    qkvp = tc.alloc_tile_pool(name="qkvp", bufs=1, side="right")
    ascr = tc.alloc_tile_pool(name="ascr", bufs=1, side="right")
    xtp = tc.alloc_tile_pool(name="xtp", bufs=1, side="right")
    wqkv = tc.alloc_tile_pool(name="wqkv", bufs=1, side="right")

    # ---------------- constants ----------------
    ones1 = consts.tile([1, P], BF16)
    nc.vector.memset(ones1, 1.0)
    ident = consts.tile([P, P], BF16)
    make_identity(nc, ident)
    b2r = consts.tile([1, D], BF16)
    nc.sync.dma_start(b2r, io["b2r"])
    b1t = consts.tile([P, NF], F32)
    nc.sync.dma_start(b1t, io["b1t"])
    ln2w = consts.tile([P, D], F32, tag="ln2w", name="ln2w_bc")
    nc.gpsimd.dma_start(ln2w, io["ln2w"].broadcast_to([P, D]))
    ln2b = consts.tile([P, D], F32, tag="ln2b", name="ln2b_bc")
    nc.gpsimd.dma_start(ln2b, io["ln2b"].broadcast_to([P, D]))
    wo_sb = []
    for k in range(ND):
        t = consts.tile([P, D], BF16, tag=f"wo{k}", name=f"wo{k}_sb")
        wo_sb.append(t)

    # ---------------- SBUF tensors ----------------
    oT = [oTp.tile([P, T], BF16, tag=f"oT{j}", name=f"oT{j}") for j in range(ND)]
    qT = [qkvp.tile([P, T], BF16, tag=f"qT{j}", name=f"qT{j}") for j in range(ND)]
    kT = [qkvp.tile([P, T], BF16, tag=f"kT{j}", name=f"kT{j}") for j in range(ND)]
    vN = [qkvp.tile([P, VW], BF16, tag=f"v{i}", name=f"v{i}") for i in range(NT)]
    xt = [xtp.tile([P, T], BF16, tag=f"xt{j}", name=f"xt{j}") for j in range(ND)]

    # weight DMAs (sync ring, consumption order: wv, wq, wk, wo)
    wv_sb, wq_sb, wk_sb = [], [], []
    for lst, name in ((wv_sb, "wv"), (wq_sb, "wq"), (wk_sb, "wk")):
        for k in range(ND):
            t = wqkv.tile([P, D], BF16, tag=f"{name}{k}", name=f"{name}{k}_sb")
            lst.append(t)
    for k in range(ND):
        nc.sync.dma_start(wv_sb[k], io["wv"][k * P:(k + 1) * P, :])
    # xT loads (scalar ring), column-group order so v can start early
    for g in range(NB):
        for k in range(ND):
            nc.scalar.dma_start(xt[k][:, g * SEQ:(g + 1) * SEQ],
                                io["xt"][k * P:(k + 1) * P, g * SEQ:(g + 1) * SEQ])
    for k in range(ND):
        nc.sync.dma_start(wq_sb[k], io["wq"][k * P:(k + 1) * P, :])
    for k in range(ND):
        nc.sync.dma_start(wk_sb[k], io["wk"][k * P:(k + 1) * P, :])
    for k in range(ND):
        nc.sync.dma_start(wo_sb[k], io["wo"][k * P:(k + 1) * P, :])

    # attention scratch tiles are allocated per-(m,b) below (tag rotation)

    # ---------------- phase V: v projection ----------------
    for i in range(NT):
        pa = psp.tile([P, SEQ], F32, tag="sc", bufs=4, name="psv_a")
        pb = psp.tile([P, SEQ], F32, tag="qk", bufs=2, name="psv_b")
        for k in range(ND):
            lhs = xt[k][:, i * P:(i + 1) * P]
            nc.tensor.matmul(pa, lhs, wv_sb[k][:, :SEQ],
                             start=(k == 0), stop=(k == ND - 1))
            nc.tensor.matmul(pb[:, 0:D - SEQ], lhs, wv_sb[k][:, SEQ:D],
                             start=(k == 0), stop=(k == ND - 1))
        nc.vector.memset(vN[i][:, HD::HD + 1], 1.0)
        vv = vN[i].rearrange("p (h w) -> p h w", w=HD + 1)
        nc.any.tensor_copy(
            out=vv[:, 0:8, 0:HD],
            in_=pa.rearrange("p (h w) -> p h w", w=HD))
        nc.any.tensor_copy(
            out=vv[:, 8:12, 0:HD],
            in_=pb[:, 0:D - SEQ].rearrange("p (h w) -> p h w", w=HD))

    lnscr = None
    w1p = None
    w1_sb = []

    def attn_scores(mm, b):
        """Score matmuls + exp for head pair (2mm, 2mm+1), batch b.
        Returns the pt tiles. Scores are 2-head row-tiled (K=64 pairs at
        row groups 0-1 / 2-3, issued back-to-back -> concurrent)."""
        bs = slice(b * SEQ, (b + 1) * SEQ)
        q0 = qT[mm][0:HD, bs]
        q1 = qT[mm][HD:P, bs]
        pts = [[], []]
        for c in range(4):
            s0 = psp.tile([P, SEQ], F32, tag="sc", bufs=4, name="st0")
            s1 = psp.tile([P, SEQ], F32, tag="sc", bufs=4, name="st1")
            kc = slice(b * SEQ + c * P, b * SEQ + (c + 1) * P)
            nc.tensor.matmul(s0, kT[mm][0:HD, kc], q0, start=True, stop=True)
            nc.tensor.matmul(s1, kT[mm][HD:P, kc], q1, start=True, stop=True)
            pt0 = ascr.tile([P, SEQ], BF16, tag=f"pt0_{c}", bufs=2,
                            name=f"pt0_{c}")
            pt1 = ascr.tile([P, SEQ], BF16, tag=f"pt1_{c}", bufs=2,
                            name=f"pt1_{c}")
            nc.scalar.activation(out=pt0, in_=s0, func=AF.Exp)
            nc.scalar.activation(out=pt1, in_=s1, func=AF.Exp)
            pts[0].append(pt0)
            pts[1].append(pt1)
        return pts

    def attn_av(mm, b, pts):
        """attn@v + denominator divide for head pair (2mm, 2mm+1), batch b."""
        bs = slice(b * SEQ, (b + 1) * SEQ)
        rb_d = dramp.tile([2, SEQ], BF16, tag="rb_d", bufs=3, name="rb_d")
        for h2 in range(2):
            h = 2 * mm + h2
            hoff = h2 * HD
            ot = psp.tile([HD + 1, SEQ], F32, tag="oc", bufs=2, name="ot")
            for c in range(4):
                vblk = vN[b * 4 + c][:, h * (HD + 1):(h + 1) * (HD + 1)]
                nc.tensor.matmul(ot, vblk, pts[h2][c],
                                 start=(c == 0), stop=(c == 3))
            nc.vector.tensor_copy(out=oT[mm][hoff:hoff + HD, bs],
                                  in_=ot[0:HD, :])
            dn = ascr.tile([1, SEQ], F32, tag=f"dn{h2}", bufs=2,
                           name=f"dn{h2}")
            nc.vector.tensor_copy(out=dn, in_=ot[HD:HD + 1, :])
            nc.vector.reciprocal(out=dn, in_=dn)
            rdb = ascr.tile([1, SEQ], BF16, tag=f"rdb{h2}", bufs=2,
                            name=f"rdb{h2}")
            nc.vector.tensor_copy(out=rdb, in_=dn)
            nc.scalar.dma_start(rb_d[h2:h2 + 1, :], rdb)
        # partition-broadcast of both heads' reciprocals via DRAM bounce
        bcs = ascr.tile([P, SEQ], BF16, tag="bcs", bufs=2, name="bcs")
        nc.scalar.dma_start(bcs[0:HD, :],
                            rb_d[0:1, :].broadcast_to([HD, SEQ]))
        nc.scalar.dma_start(bcs[HD:P, :],
                            rb_d[1:2, :].broadcast_to([HD, SEQ]))
        nc.vector.tensor_mul(out=oT[mm][:, bs], in0=oT[mm][:, bs], in1=bcs)

    pending = []  # deferred attn_av work: (mm, b, pts)

    def flush_pending():
        while pending:
            mm, b, pts = pending.pop(0)
            attn_av(mm, b, pts)

    def attention(mm):
        """Heads 2mm, 2mm+1, all batches, software-pipelined so the PE's
        attn@v for batch b never queues ahead of batch b+1's scores.
        The last batch's attn@v is DEFERRED to after the next qk block so
        its exp has a full matmul block to hide behind."""
        for b in range(NB):
            pts = attn_scores(mm, b)
            if b > 0:
                attn_av(mm, b - 1, pts_prev)
            pts_prev = pts
        pending.append((mm, NB - 1, pts_prev))

    # ---------------- merged q/k projections + attention ----------------
    xin_tiles = []
    for m in range(ND):
        for w_sb, dstT in ((wq_sb, qT), (wk_sb, kT)):
            for gp in range(2):
                pg = [psp.tile([P, SEQ], F32, tag="qk", bufs=2, name=f"pq{g}")
                      for g in range(2)]
                for k in range(ND):
                    lhs = w_sb[k][:, m * P:(m + 1) * P]
                    for g in range(2):
                        gg = gp * 2 + g
                        nc.tensor.matmul(
                            pg[g], lhs, xt[k][:, gg * SEQ:(gg + 1) * SEQ],
                            start=(k == 0), stop=(k == ND - 1))
                for g in range(2):
                    gg = gp * 2 + g
                    nc.scalar.copy(
                        out=dstT[m][:, gg * SEQ:(gg + 1) * SEQ], in_=pg[g])
        if m == ND - 1:
            # q/k weights + xt dead after this m's matmuls: free for w1
            wqkv.release()
            xtp.release()
            xinp = tc.alloc_tile_pool(name="xinp", bufs=1)
            for i in range(4):
                x_t = xinp.tile([P, D], F32, tag="xin", bufs=4, name="x_t")
                nc.sync.dma_start(x_t, io["x"][i * P:(i + 1) * P, :])
                xin_tiles.append(x_t)
            w1p = tc.alloc_tile_pool(name="w1p", bufs=1)
            for k in range(ND):
                t = w1p.tile([P, FF], BF16, tag=f"w1_{k}", name=f"w1_{k}")
                w1_sb.append(t)
            for k in range(ND):
                nc.sync.dma_start(w1_sb[k], io["w1"][k * P:(k + 1) * P, :])
        if m > 0:
            flush_pending()
            attention(m - 1)
    flush_pending()
    attention(ND - 1)
    flush_pending()

    ascr.release()
    qkvp.release()

    # LN scratch (shared by LN1 and LN2 -- the phases are sequential)
    lnscr = tc.alloc_tile_pool(name="lnscr", bufs=1)
    ln1w = lnscr.tile([P, D], F32, tag="ln1w", name="ln1w_bc")
    nc.gpsimd.dma_start(ln1w, io["ln1w"].broadcast_to([P, D]))
    ln1b = lnscr.tile([P, D], F32, tag="ln1b", name="ln1b_bc")
    nc.gpsimd.dma_start(ln1b, io["ln1b"].broadcast_to([P, D]))
    rs = {
        "veps": lnscr.tile([P, 4], F32, tag="rs_veps", bufs=2, name="rs_veps"),
        "ti": lnscr.tile([P, 4], I32, tag="rs_ti", bufs=2, name="rs_ti"),
        "yi": lnscr.tile([P, 4], I32, tag="rs_yi", bufs=2, name="rs_yi"),
        "ya": lnscr.tile([P, 4], F32, tag="rs_ya", bufs=2, name="rs_ya"),
        "p": lnscr.tile([P, 4], F32, tag="rs_p", bufs=2, name="rs_p"),
    }

    out1p = tc.alloc_tile_pool(name="out1p", bufs=1)
    o1Tp = tc.alloc_tile_pool(name="o1Tp", bufs=1)
    out1_sb = [out1p.tile([P, D], BF16, tag=f"o1_{i}", name=f"o1_{i}")
               for i in range(NT)]
    o1T = [o1Tp.tile([P, T], BF16, tag=f"o1T{j}", name=f"o1T{j}")
           for j in range(ND)]

    def ln_block(pa, pb, mvb, j):
        """bn stats for one [P, D] psum pair into mvb[:, 2j:2j+2]; STT apply
        is done separately after the batched rsqrt."""
        stats = lnscr.tile([P, 3, 6], F32, tag="stats", bufs=2, name="stats")
        nc.vector.bn_stats(out=stats[:, 0, :], in_=pa[:, 0:256])
        nc.vector.bn_stats(out=stats[:, 1, :], in_=pa[:, 256:512])
        nc.vector.bn_stats(out=stats[:, 2, :], in_=pb[:, 0:256])
        nc.vector.bn_aggr(out=mvb[:, 2 * j:2 * j + 2], in_=stats)

    def ln_apply(srcs, w_bc, base, mean_ap, rstd_ap, out_tile):
        """srcs: list of (ap, lo, hi) covering columns [0, D)."""
        u = lnscr.tile([P, D], F32, tag="u", bufs=2, name="u")
        for ap, lo, hi in srcs:
            nc.vector.scalar_tensor_tensor(
                out=u[:, lo:hi], in0=ap, scalar=mean_ap, in1=w_bc[:, lo:hi],
                op0=OP.subtract, op1=OP.mult)
        nc.vector.scalar_tensor_tensor(
            out=out_tile, in0=u, scalar=rstd_ap, in1=base,
            op0=OP.mult, op1=OP.add)

    # ---------------- Wo + LN1 (per batch, LN sub-batched by 2) ----------------
    mvb1 = lnscr.tile([P, 8], F32, tag="mvb", bufs=2, name="mvb1")
    rstd1 = lnscr.tile([P, 4], F32, tag="rstd", bufs=2, name="rstd1")
    for b in range(NB):
        holds = []
        for j in range(4):
            i = 4 * b + j
            ma = psp.tile([P, SEQ], F32, tag="sc", bufs=4, name="mh_a")
            mb = psp.tile([P, SEQ], F32, tag="qk", bufs=2, name="mh_b")
            for k in range(ND):
                lhs = oT[k][:, i * P:(i + 1) * P]
                nc.tensor.matmul(ma, lhs, wo_sb[k][:, :SEQ],
                                 start=(k == 0), stop=(k == ND - 1))
                nc.tensor.matmul(mb[:, 0:D - SEQ], lhs, wo_sb[k][:, SEQ:D],
                                 start=(k == 0), stop=(k == ND - 1))
            # decouple LN from PSUM: copy to SBUF so the psum slots free fast
            mhs = lnscr.tile([P, D], F32, tag="mhs", bufs=2, name="mhs")
            nc.vector.tensor_copy(out=mhs[:, 0:SEQ], in_=ma)
            nc.vector.tensor_copy(out=mhs[:, SEQ:D], in_=mb[:, 0:D - SEQ])
            if i >= 4:
                x_t = xinp.tile([P, D], F32, tag="xin", bufs=4, name="x_t")
                nc.sync.dma_start(x_t, io["x"][i * P:(i + 1) * P, :])
                xin_tiles.append(x_t)
            xb = lnscr.tile([P, D], F32, tag="resb", bufs=2, name="xb")
            nc.gpsimd.tensor_add(out=xb, in0=xin_tiles[i], in1=ln1b)
            stats = lnscr.tile([P, 3, 6], F32, tag="stats", bufs=2, name="stats")
            nc.vector.bn_stats(out=stats[:, 0, :], in_=mhs[:, 0:256])
            nc.vector.bn_stats(out=stats[:, 1, :], in_=mhs[:, 256:512])
            nc.vector.bn_stats(out=stats[:, 2, :], in_=mhs[:, 512:768])
            nc.vector.bn_aggr(out=mvb1[:, 2 * j:2 * j + 2], in_=stats)
            holds.append((i, j, mhs, xb))
            if j % 2 == 1:
                _rsqrt_dve(nc, rs, mvb1[:, 2 * (j - 1) + 1:2 * j + 2:2],
                           rstd1[:, j - 1:j + 1], 2)
                for (ii, jj, pmhs, pxb) in holds:
                    ln_apply([(pmhs, 0, D)], ln1w, pxb,
                             mvb1[:, 2 * jj:2 * jj + 1],
                             rstd1[:, jj:jj + 1], out1_sb[ii])
                    # transpose out1 chunk into o1T via the PE
                    for jf in range(ND):
                        ptile = psp.tile([P, P], BF16, tag="oc", bufs=2,
                                         name="ptile")
                        nc.tensor.transpose(
                            ptile, out1_sb[ii][:, jf * P:(jf + 1) * P], ident)
                        nc.any.tensor_copy(
                            out=o1T[jf][:, ii * P:(ii + 1) * P], in_=ptile)
                holds = []

    oTp.release()

    # w2 + hidden buffers
    w2p = tc.alloc_tile_pool(name="w2p", bufs=1, side="right")
    w2_sb = [w2p.tile([P, D], BF16, tag=f"w2_{k}", name=f"w2_{k}")
             for k in range(NF)]
    for k in range(NF):
        nc.sync.dma_start(w2_sb[k], io["w2"][k * P:(k + 1) * P, :])
    hbuf = tc.alloc_tile_pool(name="hbuf", bufs=1, side="right")

    # ---------------- FFN + LN2 (per batch) ----------------
    mvb2 = lnscr.tile([P, 8], F32, tag="mvb", bufs=2, name="mvb2")
    rstd2 = lnscr.tile([P, 4], F32, tag="rstd", bufs=2, name="rstd2")
    for b in range(NB):
        gs = slice(b * SEQ, (b + 1) * SEQ)
        hts = []
        for f in range(NF):
            hp = psp.tile([P, SEQ], F32, tag="sc", bufs=4, name="hp")
            for k in range(ND):
                nc.tensor.matmul(hp, w1_sb[k][:, f * P:(f + 1) * P],
                                 o1T[k][:, gs],
                                 start=(k == 0), stop=(k == ND - 1))
            ht = hbuf.tile([P, SEQ], BF16, tag=f"ht{f}", name=f"ht{f}")
            nc.scalar.activation(out=ht, in_=hp, func=AF.Gelu,
                                 bias=b1t[:, f:f + 1], scale=1.0)
            hts.append(ht)
        holds = []
        for j in range(4):
            i = 4 * b + j
            fa = psp.tile([P, SEQ], F32, tag="sc", bufs=4, name="fp_a")
            fb = psp.tile([P, SEQ], F32, tag="qk", bufs=2, name="fp_b")
            for f in range(NF):
                lhs = hts[f][:, j * P:(j + 1) * P]
                nc.tensor.matmul(fa, lhs, w2_sb[f][:, :SEQ],
                                 start=(f == 0), stop=False)
                nc.tensor.matmul(fb[:, 0:D - SEQ], lhs, w2_sb[f][:, SEQ:D],
                                 start=(f == 0), stop=False)
            nc.tensor.matmul(fa, ones1, b2r[:, :SEQ], start=False, stop=True)
            nc.tensor.matmul(fb[:, 0:D - SEQ], ones1, b2r[:, SEQ:D],
                             start=False, stop=True)
            base = lnscr.tile([P, D], F32, tag="resb", bufs=2, name="base")
            nc.gpsimd.tensor_add(out=base, in0=out1_sb[i], in1=ln2b)
            ln_block(fa, fb, mvb2, j)
            holds.append((i, j, fa, fb, base))
            if j % 2 == 1:
                _rsqrt_dve(nc, rs, mvb2[:, 2 * (j - 1) + 1:2 * j + 2:2],
                           rstd2[:, j - 1:j + 1], 2)
                for (ii, jj, pfa, pfb, pbase) in holds:
                    outt = lnscr.tile([P, D], F32, tag="outt", bufs=2,
                                      name="outt")
                    ln_apply([(pfa, 0, SEQ), (pfb[:, 0:D - SEQ], SEQ, D)],
                             ln2w, pbase, mvb2[:, 2 * jj:2 * jj + 1],
                             rstd2[:, jj:jj + 1], outt)
                    nc.scalar.dma_start(io["out"][ii * P:(ii + 1) * P, :], outt)
                holds = []

    hbuf.release()
    w2p.release()
    o1Tp.release()
    out1p.release()
    lnscr.release()
    w1p.release()
    xinp.release()
    consts.release()
    psp.release()
    dramp.release()


def build(bpc):
    """Build + compile the per-core program. Returns the Bacc object."""
    T = bpc * SEQ
    nc = bacc.Bacc("TRN2", target_bir_lowering=False, debug=False,
                   num_devices=N_CORES)
    io = {
        "x": nc.dram_tensor("x", [T, D], F32, kind="ExternalInput").ap(),
        "xt": nc.dram_tensor("xt", [D, T], BF16, kind="ExternalInput").ap(),
        "wq": nc.dram_tensor("wq", [D, D], BF16, kind="ExternalInput").ap(),
        "wk": nc.dram_tensor("wk", [D, D], BF16, kind="ExternalInput").ap(),
        "wv": nc.dram_tensor("wv", [D, D], BF16, kind="ExternalInput").ap(),
        "wo": nc.dram_tensor("wo", [D, D], BF16, kind="ExternalInput").ap(),
        "w1": nc.dram_tensor("w1", [D, FF], BF16, kind="ExternalInput").ap(),
        "w2": nc.dram_tensor("w2", [FF, D], BF16, kind="ExternalInput").ap(),
        "b1t": nc.dram_tensor("b1t", [P, NF], F32, kind="ExternalInput").ap(),
        "b2r": nc.dram_tensor("b2r", [1, D], BF16, kind="ExternalInput").ap(),
        "ln1w": nc.dram_tensor("ln1w", [1, D], F32, kind="ExternalInput").ap(),
        "ln1b": nc.dram_tensor("ln1b", [1, D], F32, kind="ExternalInput").ap(),
        "ln2w": nc.dram_tensor("ln2w", [1, D], F32, kind="ExternalInput").ap(),
        "ln2b": nc.dram_tensor("ln2b", [1, D], F32, kind="ExternalInput").ap(),
        "out": nc.dram_tensor("out", [T, D], F32, kind="ExternalOutput").ap(),
    }
    with tile.TileContext(nc) as tc:
        emit(nc, tc, io, bpc)
    nc.compile()
    return nc


def prep_weights(inputs):
    """Host-side weight layout prep (numpy only)."""
    bf = mybir.dt.np(BF16)
    f32 = np.float32
    wq = (np.asarray(inputs["Wq"], f32).transpose(1, 0, 2).reshape(D, D)
          / np.sqrt(HD)).astype(bf)
    wk = np.asarray(inputs["Wk"], f32).transpose(1, 0, 2).reshape(D, D).astype(bf)
    wv = np.asarray(inputs["Wv"], f32).transpose(1, 0, 2).reshape(D, D).astype(bf)
    return {
        "wq": np.ascontiguousarray(wq),
        "wk": np.ascontiguousarray(wk),
        "wv": np.ascontiguousarray(wv),
        "wo": np.asarray(inputs["Wo"], f32).astype(bf),
        "w1": np.asarray(inputs["W1"], f32).astype(bf),
        "w2": np.asarray(inputs["W2"], f32).astype(bf),
        "b1t": np.ascontiguousarray(
            np.asarray(inputs["b1"], f32).reshape(NF, P).T),
        "b2r": np.asarray(inputs["b2"], f32).reshape(1, D).astype(bf),
        "ln1w": np.asarray(inputs["ln1_w"], f32).reshape(1, D),
        "ln1b": np.asarray(inputs["ln1_b"], f32).reshape(1, D),
        "ln2w": np.asarray(inputs["ln2_w"], f32).reshape(1, D),
        "ln2b": np.asarray(inputs["ln2_b"], f32).reshape(1, D),
    }


def make_in_maps(inputs):
    """Per-core input dicts (shards x both ways + shared weights)."""
    bf = mybir.dt.np(BF16)
    bpc = B_TOTAL // N_CORES
    w = prep_weights(inputs)
    x = np.asarray(inputs["x"], np.float32)
    in_maps = []
    for c in range(N_CORES):
        shard = np.ascontiguousarray(
            x[c * bpc:(c + 1) * bpc].reshape(bpc * SEQ, D))
        xtr = np.ascontiguousarray(shard.T.astype(bf))
        in_maps.append({"x": shard, "xt": xtr, **w})
    return in_maps


_cache = {}


def kernel(**inputs) -> np.ndarray:
    bpc = B_TOTAL // N_CORES
    if "nc" not in _cache:
        _cache["nc"] = build(bpc)
    nc = _cache["nc"]
    in_maps = make_in_maps(inputs)
    res = run_bass_kernel_spmd(nc, in_maps, list(range(N_CORES)))
    out = np.concatenate(
        [res.results[c]["out"].reshape(bpc, SEQ, D) for c in range(N_CORES)],
        axis=0)
    return np.ascontiguousarray(out.astype(np.float32))
